# revision 26
# baseline (speedup 1.0000x reference)
"""BiRWKV layer kernel for 8 Trainium2 NeuronCores.

Strategy (data-parallel over B=8, one batch element per core):
  - (channel, time) layout on chip: channels on the 128 SBUF partitions
    (C=512 -> 4 blocks), time on the free dim.
  - r/k/v projections for both directions are bf16 matmuls
    (lhsT = W block, rhs = x^T block) accumulated over 4 input-channel
    blocks into PSUM (fp32); x tiles arrive as one batched DMA per
    512-wide half ([128, 4*512] with a rearranged HBM access pattern).
  - WKV runs UNSTABILIZED: the DVE scans compute the true state from
    ek = e^k and ekv = ek*v; the u-boosted instant terms come from a
    second exp ACT (ekb = e^{k+u}, free on the Scalar engine) and one
    4x-rate tensor_scalar (ekbv = ekv * e^u), so the denominator/
    numerator assembly on the DVE is two plain bf16 tensor adds
    (2x DVE mode):
        dnm_t = den_{t -+ 1} + ekb_t
        nmr_t = num_{t -+ 1} + ekbv_t
  - sigmoid folded into the denominator: y = nmr / (dnm*(1+e^{-r})),
    er = e^{-r} from one ACT; (dnm + er*dnm) on GpSimd. Division is
    exp(-ln(.)). ALL activation functions (exp, ln, identity, copy)
    live in the single natural_log_exp table: zero table reloads.
  - scan chain buffers are [128, PW+2] bf16 with scan output at column
    offset 1: the shifted den'_{t-1} (fwd) / den'_{t+1} (bwd) reads
    land on 4-byte-aligned offsets for full-rate DVE consumption.
  - forward-direction y stays RESIDENT in SBUF (4 x [128, 4096] bf16),
    no HBM staging roundtrip.
  - Output projection consumes the (c, t) activations directly as
    matmul lhsT; result (t, c) goes PSUM -> SBUF -> HBM.
"""

import numpy as np
import ml_dtypes

B, T, C = 8, 4096, 512
TT = 512           # time tile (psum width)
CB = 4             # channel blocks
PW = 2 * TT        # pair width for SBUF-side elementwise
NP = T // PW       # 4 pairs

_CACHE = {}


def _apply_tile_patches():
    """walrus in this container rejects instructions with >1 sync wait
    ("Too many sync wait commands"). Split excess waits onto same-engine
    nop carriers, and do the same for the TileContext tail drain."""
    import concourse.tile as tile_mod
    from concourse import mybir
    from concourse.vector_clock import ScopedClock

    if getattr(tile_mod, "_wait_split_patched", False):
        return
    MAXW = 1

    _orig_add = tile_mod.TileContext._add_instruction

    def _split_add(self, inst):
        si = inst.sync_info
        if si is not None and si.on_wait and len(si.on_wait) > MAXW:
            waits = list(si.on_wait)
            k = 0
            while len(waits) > MAXW:
                chunk, waits = waits[:MAXW], waits[MAXW:]
                carrier = mybir.InstNoOp(
                    name=f"{inst.name}_wsplit{k}",
                    engine=inst.engine,
                    bass_nofuse=True,
                    sync_info=mybir.SyncInfo(on_wait=chunk, on_update=[]),
                )
                k += 1
                _orig_add(self, carrier)
            inst.sync_info = mybir.SyncInfo(
                on_wait=waits, on_update=list(si.on_update)
            )
        return _orig_add(self, inst)

    def _drain_and_barrier(self, tick_clock, wait_clock):
        drain_inst = self.nc.sync.drain()
        wait_clock.add_sem_waits(
            drain_inst.ins, ScopedClock({None: tick_clock.global_clock})
        )
        si = drain_inst.ins.sync_info
        if si is not None and si.on_wait and len(si.on_wait) > MAXW:
            waits = list(si.on_wait)
            drain_inst.ins.sync_info = mybir.SyncInfo(
                on_wait=waits[:MAXW], on_update=list(si.on_update)
            )
            rest = waits[MAXW:]
            while rest:
                chunk, rest = rest[:MAXW], rest[MAXW:]
                n = self.nc.sync.nop(nofuse=True)
                n.ins.sync_info = mybir.SyncInfo(on_wait=chunk, on_update=[])

        self.nc.all_engine_barrier()
        assert self.sems is not None
        popped = self.nc._tile_sem_poison_stack.pop()
        assert popped is self._sem_poison
        self.nc.clear_and_free_semaphores(list(self.sems.allocated().values()))
        self.nc.all_engine_barrier()

    tile_mod.TileContext._add_instruction = _split_add
    tile_mod.TileContext._drain_and_barrier = _drain_and_barrier
    tile_mod._wait_split_patched = True


def _build_nc():
    import concourse.bass as bass
    import concourse.tile as tile
    from concourse import mybir

    _apply_tile_patches()

    f32 = mybir.dt.float32
    bf16 = mybir.dt.bfloat16
    Alu = mybir.AluOpType
    Act = mybir.ActivationFunctionType

    nc = bass.Bass()

    xT = nc.dram_tensor("xT", [C, T], bf16, kind="ExternalInput")
    wnames = ["w_rf", "w_kf", "w_vf", "w_rb", "w_kb", "w_vb"]
    wdram = {
        n: nc.dram_tensor(n, [128, 4 * C], bf16, kind="ExternalInput")
        for n in wnames
    }
    wout_d = nc.dram_tensor("wout", [128, 8 * C], bf16, kind="ExternalInput")
    cst_f_d = nc.dram_tensor("cst_f", [C, 3], f32, kind="ExternalInput")
    cst_b_d = nc.dram_tensor("cst_b", [C, 3], f32, kind="ExternalInput")
    out_d = nc.dram_tensor("y", [T, C], f32, kind="ExternalOutput")

    with tile.TileContext(nc) as tc:
        with (
            tc.tile_pool(name="wp", bufs=1) as wp,
            tc.tile_pool(name="cst", bufs=1) as cst,
            tc.tile_pool(name="chain", bufs=1) as chainp,
            tc.tile_pool(name="xt", bufs=2) as xtp,
            tc.tile_pool(name="yf", bufs=1) as yfp,
            tc.tile_pool(name="wk", bufs=1) as wkp,
            tc.tile_pool(name="ps", bufs=1, space="PSUM") as psp,
        ):
            # ---- first x pair, then fwd weights & constants, then the
            # bwd/out weights (needed later; keeps the Sync DMA-issue
            # queue from delaying the critical first matmuls) ----
            xh0 = {}
            for half in range(2):
                t0 = half * TT
                xh = xtp.tile([128, 4 * TT], bf16, tag=f"xh{half}",
                              bufs=2, name=f"xh{half}")
                nc.sync.dma_start(
                    xh[:],
                    xT[0:C, t0: t0 + TT].rearrange("(kb p) c -> p kb c",
                                                   kb=4))
                xh0[half] = xh
            wt = {}
            for n in ("w_kf", "w_vf", "w_rf"):
                wt[n] = wp.tile([128, 4 * C], bf16, tag=n, name=n)
                nc.sync.dma_start(wt[n][:], wdram[n][:])
            # constants batched: one [128, 4cb*3] tile per direction
            # (cols per cb: u, eu, dec)
            u_t, eu_t, dec_t = {}, {}, {}
            for d, cd in (("f", cst_f_d), ("b", cst_b_d)):
                ct = cst.tile([128, 3 * CB], f32, tag=f"cst{d}",
                              name=f"cst{d}")
                nc.sync.dma_start(
                    ct[:], cd[:, :].rearrange("(cb p) c -> p cb c", cb=CB))
                for cb in range(CB):
                    u_t[(d, cb)] = ct[:, 3 * cb: 3 * cb + 1]
                    eu_t[(d, cb)] = ct[:, 3 * cb + 1: 3 * cb + 2]
                    dec_t[(d, cb)] = ct[:, 3 * cb + 2: 3 * cb + 3]
            for n in ("w_kb", "w_vb", "w_rb"):
                wt[n] = wp.tile([128, 4 * C], bf16, tag=n, name=n)
                nc.sync.dma_start(wt[n][:], wdram[n][:])
            wout = wp.tile([128, 8 * C], bf16, name="wout")
            nc.sync.dma_start(wout[:], wout_d[:])

            # forward y, SBUF-resident for the whole kernel
            yf_t = {}
            for cb in range(CB):
                yf_t[cb] = yfp.tile([128, T], bf16, tag=f"yf{cb}",
                                    name=f"yf{cb}")

            # scan chain buffers: [128, PW+2] bf16; scan writes cols
            # [1:1+PW]; shifted reads hit even offsets: fwd prev at
            # [0:PW], bwd next at [2:2+PW]. Boundary state in col 0
            # (fwd) / col PW+1 (bwd).
            chains = {}
            for cb in range(CB):
                for q in ("den", "num"):
                    chains[(cb, q)] = chainp.tile(
                        [128, PW + 2], bf16, tag=f"ch_{q}{cb}",
                        name=f"ch_{q}{cb}")

            def emit_outproj(p0, yp_tiles, ms=None):
                for m in ms if ms is not None else range(PW // 128):
                    t0 = p0 + m * 128
                    pso = psp.tile([128, C], f32, tag="po",
                                   bufs=2, name="pso")
                    for cb in range(CB):
                        nc.tensor.matmul(
                            pso[:],
                            yf_t[cb][:, t0: t0 + 128],
                            wout[:, cb * C: (cb + 1) * C],
                            start=(cb == 0), stop=False)
                    for cb in range(CB):
                        nc.tensor.matmul(
                            pso[:],
                            yp_tiles[cb][:, m * 128: (m + 1) * 128],
                            wout[:, (4 + cb) * C: (5 + cb) * C],
                            start=False, stop=(cb == 3))
                    osb = wkp.tile([128, C], f32, tag="osb",
                                   bufs=2, name="osb")
                    nc.scalar.copy(osb[:], pso[:])
                    nc.sync.dma_start(out_d[t0: t0 + 128, :], osb[:])

            def run_phase(d):
                fwd = d == "f"
                wr, wk, wv = wt["w_r" + d], wt["w_k" + d], wt["w_v" + d]
                pairs = list(range(NP)) if fwd else list(reversed(range(NP)))
                pending = None

                for pi, pr in enumerate(pairs):
                    p0 = pr * PW
                    # batched x DMA: one [128, 4*TT] tile per half
                    if fwd and pi == 0:
                        xhs = xh0
                    else:
                        xhs = {}
                        for half, tt in enumerate((2 * pr, 2 * pr + 1)):
                            t0 = tt * TT
                            xh = xtp.tile([128, 4 * TT], bf16,
                                          tag=f"xh{half}", bufs=2,
                                          name=f"xh{half}")
                            nc.sync.dma_start(
                                xh[:],
                                xT[0:C, t0: t0 + TT].rearrange(
                                    "(kb p) c -> p kb c", kb=4))
                            xhs[half] = xh
                    yp_tiles = {}
                    for cb in range(CB):
                        pss = {}
                        for cls, w in (("k", wk), ("v", wv), ("r", wr)):
                            for half in range(2):
                                pss[(cls, half)] = psp.tile(
                                    [128, TT], f32, tag=f"p{cls}", bufs=2,
                                    name=f"ps{cls}")
                            for kb in range(4):
                                wsl = w[:, kb * C + cb * 128:
                                        kb * C + cb * 128 + 128]
                                for half in range(2):
                                    nc.tensor.matmul(
                                        pss[(cls, half)][:], wsl,
                                        xhs[half][:, kb * TT:(kb + 1) * TT],
                                        start=(kb == 0), stop=(kb == 3))
                        # last pair: flush the pending out-projection right
                        # behind the final projection matmuls so the PE
                        # tail overlaps the DVE's last elementwise work
                        if (not fwd and pi == NP - 1 and cb == CB - 1
                                and pending is not None):
                            emit_outproj(*pending)
                            pending = None
                        # per-cb full-pair (1024-wide) activations
                        ek = wkp.tile([128, PW], bf16, tag="ek", bufs=3,
                                      name="ek")
                        ekb = wkp.tile([128, PW], bf16, tag="ekb", bufs=3,
                                       name="ekb")
                        er = wkp.tile([128, PW], bf16, tag="er", bufs=3,
                                      name="er")
                        vsb = wkp.tile([128, PW], bf16, tag="vsb", bufs=3,
                                       name="vsb")
                        for half in range(2):
                            hs = slice(half * TT, (half + 1) * TT)
                            nc.scalar.activation(
                                ek[:, hs], pss[("k", half)][:], Act.Exp)
                            nc.scalar.activation(
                                ekb[:, hs], pss[("k", half)][:], Act.Exp,
                                bias=u_t[(d, cb)])
                            nc.scalar.activation(
                                er[:, hs], pss[("r", half)][:], Act.Exp,
                                scale=-1.0)
                            nc.scalar.copy(vsb[:, hs], pss[("v", half)][:])
                        ekv = wkp.tile([128, PW], bf16, tag="ekv", bufs=3,
                                       name="ekv")
                        nc.vector.tensor_mul(ekv[:], ek[:], vsb[:])
                        ekbv = wkp.tile([128, PW], bf16, tag="ekbv", bufs=3,
                                        name="ekbv")
                        nc.vector.tensor_scalar_mul(
                            ekbv[:], ekv[:], eu_t[(d, cb)])

                        # scans: den over ek, num over ekv (bf16 out)
                        decbc = dec_t[(d, cb)].broadcast_to([128, PW])
                        dnm = wkp.tile([128, PW], bf16, tag="dnm", bufs=3,
                                       name="dnm")
                        nmr = wkp.tile([128, PW], bf16, tag="nmr", bufs=3,
                                       name="nmr")
                        for q, data, addt, outb in (
                                ("den", ek, ekb, dnm),
                                ("num", ekv, ekbv, nmr)):
                            ch = chains[(cb, q)]
                            if fwd:
                                # boundary col 0 := previous pair's col PW
                                # ([128,1] housekeeping on the idle GpSimd)
                                if pi == 0:
                                    nc.vector.memset(ch[:, 0:1], 0.0)
                                else:
                                    nc.vector.tensor_copy(
                                        ch[:, 0:1], ch[:, PW: PW + 1])
                                nc.vector.tensor_tensor_scan(
                                    ch[:, 1: 1 + PW], decbc, data[:],
                                    ch[:, 0:1], Alu.mult, Alu.add)
                                prev = ch[:, 0:PW]
                            else:
                                if pi == 0:
                                    nc.gpsimd.memset(
                                        ch[:, PW + 1: PW + 2], 0.0)
                                else:
                                    nc.vector.tensor_copy(
                                        ch[:, PW + 1: PW + 2], ch[:, 1:2])
                                nc.vector.tensor_tensor_scan(
                                    ch[:, 1: 1 + PW][:, ::-1], decbc,
                                    data[:][:, ::-1],
                                    ch[:, PW + 1: PW + 2],
                                    Alu.mult, Alu.add)
                                prev = ch[:, 2: 2 + PW]
                            # dnm_t = den_{t -+ 1} + e^u-boosted instant term
                            nc.vector.tensor_add(outb[:], prev, addt[:])

                        # dnm2 = dnm + er * dnm   (sigmoid folded in).
                        # On the DVE, NOT GpSimd: Pool shares SBUF ports
                        # with the DVE but runs ~4x slower per element, so
                        # any Pool op steals more DVE throughput than it
                        # contributes.
                        erd = wkp.tile([128, PW], bf16, tag="erd", bufs=3,
                                       name="erd")
                        nc.vector.tensor_mul(erd[:], er[:], dnm[:])
                        dnm2 = wkp.tile([128, PW], bf16, tag="dnm2", bufs=3,
                                        name="dnm2")
                        nc.vector.tensor_add(dnm2[:], dnm[:], erd[:])
                        lnb = wkp.tile([128, PW], f32, tag="lnb", bufs=2,
                                       name="lnb")
                        nc.scalar.activation(lnb[:], dnm2[:], Act.Ln)
                        inv = wkp.tile([128, PW], bf16, tag="inv", bufs=3,
                                       name="inv")
                        nc.scalar.activation(inv[:], lnb[:], Act.Exp,
                                             scale=-1.0)
                        if fwd:
                            nc.vector.tensor_mul(
                                yf_t[cb][:, p0: p0 + PW], nmr[:], inv[:])
                        elif pi == NP - 1:
                            # last pair: y in two halves so the final
                            # out-projection pipelines behind each half
                            yb = wkp.tile([128, PW], bf16, tag=f"yp{cb}",
                                          bufs=2, name=f"yp{cb}")
                            for half in range(2):
                                hs = slice(half * TT, (half + 1) * TT)
                                nc.vector.tensor_mul(yb[:, hs], nmr[:, hs],
                                                     inv[:, hs])
                            yp_tiles[cb] = yb
                        else:
                            yb = wkp.tile([128, PW], bf16, tag=f"yp{cb}",
                                          bufs=2, name=f"yp{cb}")
                            nc.vector.tensor_mul(yb[:], nmr[:], inv[:])
                            yp_tiles[cb] = yb

                    # out-projection (bwd only), deferred by one pair so
                    # its matmuls sit BEHIND the next pair's projection
                    # matmuls in the PE queue and never starve the DVE
                    if not fwd:
                        if pending is not None:
                            emit_outproj(*pending)
                        pending = (p0, yp_tiles)
                if pending is not None:
                    p0l, ypl = pending
                    emit_outproj(p0l, ypl, ms=range(0, PW // 256))
                    emit_outproj(p0l, ypl, ms=range(PW // 256, PW // 128))

            run_phase("f")
            run_phase("b")

    return nc


def _host_prep(x, W_rkv, W_out, time_decay, time_first, time_decay_rev,
               time_first_rev):
    bf16 = ml_dtypes.bfloat16
    f32 = np.float32

    Wr = W_rkv.reshape(C, 2, 3, C)
    pieces = {
        "w_rf": Wr[:, 0, 0], "w_kf": Wr[:, 0, 1], "w_vf": Wr[:, 0, 2],
        "w_rb": Wr[:, 1, 0], "w_kb": Wr[:, 1, 1], "w_vb": Wr[:, 1, 2],
    }
    wmaps = {}
    for n, p in pieces.items():
        wmaps[n] = np.ascontiguousarray(
            p.reshape(4, 128, C).transpose(1, 0, 2).reshape(128, 4 * C)
        ).astype(bf16)

    Wo = W_out.reshape(8, 128, C).transpose(1, 0, 2).reshape(128, 8 * C)
    wout = np.ascontiguousarray(Wo).astype(bf16)

    cst_f = np.stack([
        time_first.astype(np.float64),
        np.exp(time_first.astype(np.float64)),
        np.exp(-np.exp(time_decay.astype(np.float64))),
    ], axis=1).astype(f32)
    cst_b = np.stack([
        time_first_rev.astype(np.float64),
        np.exp(time_first_rev.astype(np.float64)),
        np.exp(-np.exp(time_decay_rev.astype(np.float64))),
    ], axis=1).astype(f32)

    shared = dict(wout=wout, cst_f=np.ascontiguousarray(cst_f),
                  cst_b=np.ascontiguousarray(cst_b), **wmaps)
    in_maps = []
    for b in range(B):
        m = dict(shared)
        m["xT"] = np.ascontiguousarray(x[b].T).astype(bf16)
        in_maps.append(m)
    return in_maps


def kernel(x, W_rkv, W_out, time_decay, time_first, time_decay_rev,
           time_first_rev, _trace=False):
    from concourse.bass_utils import run_bass_kernel_spmd

    x = np.asarray(x, dtype=np.float32)
    W_rkv = np.asarray(W_rkv, dtype=np.float32)
    W_out = np.asarray(W_out, dtype=np.float32)
    time_decay = np.asarray(time_decay, dtype=np.float32)
    time_first = np.asarray(time_first, dtype=np.float32)
    time_decay_rev = np.asarray(time_decay_rev, dtype=np.float32)
    time_first_rev = np.asarray(time_first_rev, dtype=np.float32)

    if "nc" not in _CACHE:
        _CACHE["nc"] = _build_nc()
    nc = _CACHE["nc"]

    in_maps = _host_prep(x, W_rkv, W_out, time_decay, time_first,
                         time_decay_rev, time_first_rev)
    res = run_bass_kernel_spmd(
        nc, in_maps, core_ids=list(range(B)), trace=_trace
    )
    _CACHE["last_result"] = res
    out = np.stack([res.results[b]["y"].astype(np.float32) for b in range(B)])
    return out


# revision 29
# speedup vs baseline: 1.0006x; 1.0006x over previous
"""BiRWKV layer kernel for 8 Trainium2 NeuronCores.

Strategy (data-parallel over B=8, one batch element per core):
  - (channel, time) layout on chip: channels on the 128 SBUF partitions
    (C=512 -> 4 blocks), time on the free dim.
  - r/k/v projections for both directions are bf16 matmuls
    (lhsT = W block, rhs = x^T block) accumulated over 4 input-channel
    blocks into PSUM (fp32); x tiles arrive as one batched DMA per
    512-wide half ([128, 4*512] with a rearranged HBM access pattern).
  - WKV runs UNSTABILIZED: the DVE scans compute the true state from
    ek = e^k and ekv = ek*v; the u-boosted instant terms come from a
    second exp ACT (ekb = e^{k+u}, free on the Scalar engine) and one
    4x-rate tensor_scalar (ekbv = ekv * e^u), so the denominator/
    numerator assembly on the DVE is two plain bf16 tensor adds
    (2x DVE mode):
        dnm_t = den_{t -+ 1} + ekb_t
        nmr_t = num_{t -+ 1} + ekbv_t
  - sigmoid folded into the denominator: y = nmr / (dnm*(1+e^{-r})),
    er = e^{-r} from one ACT; (dnm + er*dnm) on GpSimd. Division is
    exp(-ln(.)). ALL activation functions (exp, ln, identity, copy)
    live in the single natural_log_exp table: zero table reloads.
  - scan chain buffers are [128, PW+2] bf16 with scan output at column
    offset 1: the shifted den'_{t-1} (fwd) / den'_{t+1} (bwd) reads
    land on 4-byte-aligned offsets for full-rate DVE consumption.
  - forward-direction y stays RESIDENT in SBUF (4 x [128, 4096] bf16),
    no HBM staging roundtrip.
  - Output projection consumes the (c, t) activations directly as
    matmul lhsT; result (t, c) goes PSUM -> SBUF -> HBM.
"""

import numpy as np
import ml_dtypes

B, T, C = 8, 4096, 512
TT = 512           # time tile (psum width)
CB = 4             # channel blocks
PW = 2 * TT        # pair width for SBUF-side elementwise
NP = T // PW       # 4 pairs

_CACHE = {}


def _apply_tile_patches():
    """walrus in this container rejects instructions with >1 sync wait
    ("Too many sync wait commands"). Split excess waits onto same-engine
    nop carriers, and do the same for the TileContext tail drain."""
    import concourse.tile as tile_mod
    from concourse import mybir
    from concourse.vector_clock import ScopedClock

    if getattr(tile_mod, "_wait_split_patched", False):
        return
    MAXW = 1

    _orig_add = tile_mod.TileContext._add_instruction

    def _split_add(self, inst):
        si = inst.sync_info
        if si is not None and si.on_wait and len(si.on_wait) > MAXW:
            waits = list(si.on_wait)
            k = 0
            while len(waits) > MAXW:
                chunk, waits = waits[:MAXW], waits[MAXW:]
                carrier = mybir.InstNoOp(
                    name=f"{inst.name}_wsplit{k}",
                    engine=inst.engine,
                    bass_nofuse=True,
                    sync_info=mybir.SyncInfo(on_wait=chunk, on_update=[]),
                )
                k += 1
                _orig_add(self, carrier)
            inst.sync_info = mybir.SyncInfo(
                on_wait=waits, on_update=list(si.on_update)
            )
        return _orig_add(self, inst)

    def _drain_and_barrier(self, tick_clock, wait_clock):
        drain_inst = self.nc.sync.drain()
        wait_clock.add_sem_waits(
            drain_inst.ins, ScopedClock({None: tick_clock.global_clock})
        )
        si = drain_inst.ins.sync_info
        if si is not None and si.on_wait and len(si.on_wait) > MAXW:
            waits = list(si.on_wait)
            drain_inst.ins.sync_info = mybir.SyncInfo(
                on_wait=waits[:MAXW], on_update=list(si.on_update)
            )
            rest = waits[MAXW:]
            while rest:
                chunk, rest = rest[:MAXW], rest[MAXW:]
                n = self.nc.sync.nop(nofuse=True)
                n.ins.sync_info = mybir.SyncInfo(on_wait=chunk, on_update=[])

        self.nc.all_engine_barrier()
        assert self.sems is not None
        popped = self.nc._tile_sem_poison_stack.pop()
        assert popped is self._sem_poison
        self.nc.clear_and_free_semaphores(list(self.sems.allocated().values()))
        self.nc.all_engine_barrier()

    tile_mod.TileContext._add_instruction = _split_add
    tile_mod.TileContext._drain_and_barrier = _drain_and_barrier
    tile_mod._wait_split_patched = True


def _build_nc():
    import concourse.bass as bass
    import concourse.tile as tile
    from concourse import mybir

    _apply_tile_patches()

    f32 = mybir.dt.float32
    bf16 = mybir.dt.bfloat16
    Alu = mybir.AluOpType
    Act = mybir.ActivationFunctionType

    nc = bass.Bass()

    xT = nc.dram_tensor("xT", [C, T], bf16, kind="ExternalInput")
    wnames = ["w_rf", "w_kf", "w_vf", "w_rb", "w_kb", "w_vb"]
    wdram = {
        n: nc.dram_tensor(n, [128, 4 * C], bf16, kind="ExternalInput")
        for n in wnames
    }
    wout_d = nc.dram_tensor("wout", [128, 8 * C], bf16, kind="ExternalInput")
    cst_f_d = nc.dram_tensor("cst_f", [C, 3], f32, kind="ExternalInput")
    cst_b_d = nc.dram_tensor("cst_b", [C, 3], f32, kind="ExternalInput")
    out_d = nc.dram_tensor("y", [T, C], f32, kind="ExternalOutput")

    with tile.TileContext(nc) as tc:
        with (
            tc.tile_pool(name="wp", bufs=1) as wp,
            tc.tile_pool(name="cst", bufs=1) as cst,
            tc.tile_pool(name="chain", bufs=1) as chainp,
            tc.tile_pool(name="xt", bufs=2) as xtp,
            tc.tile_pool(name="yf", bufs=1) as yfp,
            tc.tile_pool(name="wk", bufs=1) as wkp,
            tc.tile_pool(name="ps", bufs=1, space="PSUM") as psp,
        ):
            # ---- first x pair, then fwd weights & constants, then the
            # bwd/out weights (needed later; keeps the Sync DMA-issue
            # queue from delaying the critical first matmuls) ----
            xh0 = {}
            for half in range(2):
                t0 = half * TT
                xh = xtp.tile([128, 4 * TT], bf16, tag=f"xh{half}",
                              bufs=2, name=f"xh{half}")
                nc.sync.dma_start(
                    xh[:],
                    xT[0:C, t0: t0 + TT].rearrange("(kb p) c -> p kb c",
                                                   kb=4))
                xh0[half] = xh
            wt = {}
            for n in ("w_kf", "w_vf", "w_rf"):
                wt[n] = wp.tile([128, 4 * C], bf16, tag=n, name=n)
                nc.sync.dma_start(wt[n][:], wdram[n][:])
            # constants batched: one [128, 4cb*3] tile per direction
            # (cols per cb: u, eu, dec)
            u_t, eu_t, dec_t = {}, {}, {}
            for d, cd in (("f", cst_f_d), ("b", cst_b_d)):
                ct = cst.tile([128, 3 * CB], f32, tag=f"cst{d}",
                              name=f"cst{d}")
                nc.sync.dma_start(
                    ct[:], cd[:, :].rearrange("(cb p) c -> p cb c", cb=CB))
                for cb in range(CB):
                    u_t[(d, cb)] = ct[:, 3 * cb: 3 * cb + 1]
                    eu_t[(d, cb)] = ct[:, 3 * cb + 1: 3 * cb + 2]
                    dec_t[(d, cb)] = ct[:, 3 * cb + 2: 3 * cb + 3]
            for n in ("w_kb", "w_vb", "w_rb"):
                wt[n] = wp.tile([128, 4 * C], bf16, tag=n, name=n)
                nc.sync.dma_start(wt[n][:], wdram[n][:])
            wout = wp.tile([128, 8 * C], bf16, name="wout")
            nc.sync.dma_start(wout[:], wout_d[:])

            # forward y, SBUF-resident for the whole kernel
            yf_t = {}
            for cb in range(CB):
                yf_t[cb] = yfp.tile([128, T], bf16, tag=f"yf{cb}",
                                    name=f"yf{cb}")

            # scan chain buffers: [128, PW+2] bf16; scan writes cols
            # [1:1+PW]; shifted reads hit even offsets: fwd prev at
            # [0:PW], bwd next at [2:2+PW]. Boundary state in col 0
            # (fwd) / col PW+1 (bwd).
            chains = {}
            for cb in range(CB):
                for q in ("den", "num"):
                    chains[(cb, q)] = chainp.tile(
                        [128, PW + 2], bf16, tag=f"ch_{q}{cb}",
                        name=f"ch_{q}{cb}")

            def emit_outproj(p0, yp_tiles, ms=None):
                for m in ms if ms is not None else range(PW // 128):
                    t0 = p0 + m * 128
                    pso = psp.tile([128, C], f32, tag="po",
                                   bufs=2, name="pso")
                    for cb in range(CB):
                        nc.tensor.matmul(
                            pso[:],
                            yf_t[cb][:, t0: t0 + 128],
                            wout[:, cb * C: (cb + 1) * C],
                            start=(cb == 0), stop=False)
                    for cb in range(CB):
                        nc.tensor.matmul(
                            pso[:],
                            yp_tiles[cb][:, m * 128: (m + 1) * 128],
                            wout[:, (4 + cb) * C: (5 + cb) * C],
                            start=False, stop=(cb == 3))
                    osb = wkp.tile([128, C], f32, tag="osb",
                                   bufs=4, name="osb")
                    nc.scalar.copy(osb[:], pso[:])
                    nc.sync.dma_start(out_d[t0: t0 + 128, :], osb[:])

            def run_phase(d):
                fwd = d == "f"
                wr, wk, wv = wt["w_r" + d], wt["w_k" + d], wt["w_v" + d]
                pairs = list(range(NP)) if fwd else list(reversed(range(NP)))
                pending = None

                for pi, pr in enumerate(pairs):
                    p0 = pr * PW
                    # batched x DMA: one [128, 4*TT] tile per half
                    if fwd and pi == 0:
                        xhs = xh0
                    else:
                        xhs = {}
                        for half, tt in enumerate((2 * pr, 2 * pr + 1)):
                            t0 = tt * TT
                            xh = xtp.tile([128, 4 * TT], bf16,
                                          tag=f"xh{half}", bufs=2,
                                          name=f"xh{half}")
                            nc.sync.dma_start(
                                xh[:],
                                xT[0:C, t0: t0 + TT].rearrange(
                                    "(kb p) c -> p kb c", kb=4))
                            xhs[half] = xh
                    yp_tiles = {}
                    for cb in range(CB):
                        pss = {}
                        for cls, w in (("k", wk), ("v", wv), ("r", wr)):
                            for half in range(2):
                                pss[(cls, half)] = psp.tile(
                                    [128, TT], f32, tag=f"p{cls}", bufs=2,
                                    name=f"ps{cls}")
                            for kb in range(4):
                                wsl = w[:, kb * C + cb * 128:
                                        kb * C + cb * 128 + 128]
                                for half in range(2):
                                    nc.tensor.matmul(
                                        pss[(cls, half)][:], wsl,
                                        xhs[half][:, kb * TT:(kb + 1) * TT],
                                        start=(kb == 0), stop=(kb == 3))
                        # last pair: flush the pending out-projection right
                        # behind the final projection matmuls so the PE
                        # tail overlaps the DVE's last elementwise work
                        if (not fwd and pi == NP - 1 and cb == CB - 1
                                and pending is not None):
                            emit_outproj(*pending)
                            pending = None
                        # per-cb full-pair (1024-wide) activations
                        ek = wkp.tile([128, PW], bf16, tag="ek", bufs=3,
                                      name="ek")
                        ekb = wkp.tile([128, PW], bf16, tag="ekb", bufs=3,
                                       name="ekb")
                        er = wkp.tile([128, PW], bf16, tag="er", bufs=3,
                                      name="er")
                        vsb = wkp.tile([128, PW], bf16, tag="vsb", bufs=3,
                                       name="vsb")
                        for half in range(2):
                            hs = slice(half * TT, (half + 1) * TT)
                            nc.scalar.activation(
                                ek[:, hs], pss[("k", half)][:], Act.Exp)
                            nc.scalar.activation(
                                ekb[:, hs], pss[("k", half)][:], Act.Exp,
                                bias=u_t[(d, cb)])
                            nc.scalar.activation(
                                er[:, hs], pss[("r", half)][:], Act.Exp,
                                scale=-1.0)
                            nc.scalar.copy(vsb[:, hs], pss[("v", half)][:])
                        ekv = wkp.tile([128, PW], bf16, tag="ekv", bufs=3,
                                       name="ekv")
                        nc.vector.tensor_mul(ekv[:], ek[:], vsb[:])
                        ekbv = wkp.tile([128, PW], bf16, tag="ekbv", bufs=3,
                                        name="ekbv")
                        nc.vector.tensor_scalar_mul(
                            ekbv[:], ekv[:], eu_t[(d, cb)])

                        # scans: den over ek, num over ekv (bf16 out)
                        decbc = dec_t[(d, cb)].broadcast_to([128, PW])
                        dnm = wkp.tile([128, PW], bf16, tag="dnm", bufs=3,
                                       name="dnm")
                        nmr = wkp.tile([128, PW], bf16, tag="nmr", bufs=3,
                                       name="nmr")
                        for q, data, addt, outb in (
                                ("den", ek, ekb, dnm),
                                ("num", ekv, ekbv, nmr)):
                            ch = chains[(cb, q)]
                            if fwd:
                                # boundary col 0 := previous pair's col PW
                                # ([128,1] housekeeping on the idle GpSimd)
                                if pi == 0:
                                    nc.vector.memset(ch[:, 0:1], 0.0)
                                else:
                                    nc.vector.tensor_copy(
                                        ch[:, 0:1], ch[:, PW: PW + 1])
                                nc.vector.tensor_tensor_scan(
                                    ch[:, 1: 1 + PW], decbc, data[:],
                                    ch[:, 0:1], Alu.mult, Alu.add)
                                prev = ch[:, 0:PW]
                            else:
                                if pi == 0:
                                    nc.gpsimd.memset(
                                        ch[:, PW + 1: PW + 2], 0.0)
                                else:
                                    nc.vector.tensor_copy(
                                        ch[:, PW + 1: PW + 2], ch[:, 1:2])
                                nc.vector.tensor_tensor_scan(
                                    ch[:, 1: 1 + PW][:, ::-1], decbc,
                                    data[:][:, ::-1],
                                    ch[:, PW + 1: PW + 2],
                                    Alu.mult, Alu.add)
                                prev = ch[:, 2: 2 + PW]
                            # dnm_t = den_{t -+ 1} + e^u-boosted instant term
                            nc.vector.tensor_add(outb[:], prev, addt[:])

                        # dnm2 = dnm + er * dnm   (sigmoid folded in).
                        # On the DVE, NOT GpSimd: Pool shares SBUF ports
                        # with the DVE but runs ~4x slower per element, so
                        # any Pool op steals more DVE throughput than it
                        # contributes.
                        erd = wkp.tile([128, PW], bf16, tag="erd", bufs=3,
                                       name="erd")
                        nc.vector.tensor_mul(erd[:], er[:], dnm[:])
                        dnm2 = wkp.tile([128, PW], bf16, tag="dnm2", bufs=3,
                                        name="dnm2")
                        nc.vector.tensor_add(dnm2[:], dnm[:], erd[:])
                        lnb = wkp.tile([128, PW], f32, tag="lnb", bufs=2,
                                       name="lnb")
                        nc.scalar.activation(lnb[:], dnm2[:], Act.Ln)
                        inv = wkp.tile([128, PW], bf16, tag="inv", bufs=3,
                                       name="inv")
                        nc.scalar.activation(inv[:], lnb[:], Act.Exp,
                                             scale=-1.0)
                        if fwd:
                            nc.vector.tensor_mul(
                                yf_t[cb][:, p0: p0 + PW], nmr[:], inv[:])
                        else:
                            yb = wkp.tile([128, PW], bf16, tag=f"yp{cb}",
                                          bufs=2, name=f"yp{cb}")
                            nc.vector.tensor_mul(yb[:], nmr[:], inv[:])
                            yp_tiles[cb] = yb

                    # out-projection (bwd only), deferred by one pair so
                    # its matmuls sit BEHIND the next pair's projection
                    # matmuls in the PE queue and never starve the DVE
                    if not fwd:
                        if pending is not None:
                            emit_outproj(*pending)
                        pending = (p0, yp_tiles)
                if pending is not None:
                    emit_outproj(*pending)

            run_phase("f")
            run_phase("b")

    return nc


def _host_prep(x, W_rkv, W_out, time_decay, time_first, time_decay_rev,
               time_first_rev):
    bf16 = ml_dtypes.bfloat16
    f32 = np.float32

    Wr = W_rkv.reshape(C, 2, 3, C)
    pieces = {
        "w_rf": Wr[:, 0, 0], "w_kf": Wr[:, 0, 1], "w_vf": Wr[:, 0, 2],
        "w_rb": Wr[:, 1, 0], "w_kb": Wr[:, 1, 1], "w_vb": Wr[:, 1, 2],
    }
    wmaps = {}
    for n, p in pieces.items():
        wmaps[n] = np.ascontiguousarray(
            p.reshape(4, 128, C).transpose(1, 0, 2).reshape(128, 4 * C)
        ).astype(bf16)

    Wo = W_out.reshape(8, 128, C).transpose(1, 0, 2).reshape(128, 8 * C)
    wout = np.ascontiguousarray(Wo).astype(bf16)

    cst_f = np.stack([
        time_first.astype(np.float64),
        np.exp(time_first.astype(np.float64)),
        np.exp(-np.exp(time_decay.astype(np.float64))),
    ], axis=1).astype(f32)
    cst_b = np.stack([
        time_first_rev.astype(np.float64),
        np.exp(time_first_rev.astype(np.float64)),
        np.exp(-np.exp(time_decay_rev.astype(np.float64))),
    ], axis=1).astype(f32)

    shared = dict(wout=wout, cst_f=np.ascontiguousarray(cst_f),
                  cst_b=np.ascontiguousarray(cst_b), **wmaps)
    in_maps = []
    for b in range(B):
        m = dict(shared)
        m["xT"] = np.ascontiguousarray(x[b].T).astype(bf16)
        in_maps.append(m)
    return in_maps


def kernel(x, W_rkv, W_out, time_decay, time_first, time_decay_rev,
           time_first_rev, _trace=False):
    from concourse.bass_utils import run_bass_kernel_spmd

    x = np.asarray(x, dtype=np.float32)
    W_rkv = np.asarray(W_rkv, dtype=np.float32)
    W_out = np.asarray(W_out, dtype=np.float32)
    time_decay = np.asarray(time_decay, dtype=np.float32)
    time_first = np.asarray(time_first, dtype=np.float32)
    time_decay_rev = np.asarray(time_decay_rev, dtype=np.float32)
    time_first_rev = np.asarray(time_first_rev, dtype=np.float32)

    if "nc" not in _CACHE:
        _CACHE["nc"] = _build_nc()
    nc = _CACHE["nc"]

    in_maps = _host_prep(x, W_rkv, W_out, time_decay, time_first,
                         time_decay_rev, time_first_rev)
    res = run_bass_kernel_spmd(
        nc, in_maps, core_ids=list(range(B)), trace=_trace
    )
    _CACHE["last_result"] = res
    out = np.stack([res.results[b]["y"].astype(np.float32) for b in range(B)])
    return out
